# revision 1
# baseline (speedup 1.0000x reference)
"""Trainium2 Bass kernel for nn_MoE_25005299597538 (moe_routing).

Strategy: expert-parallel with host-side routing (the gate is 0.01% of the
FLOPs; the reference's fp32 top-5 selection is reproduced exactly by an fp64
host gate — verified min p5/p6 gap 1.9e-6 >> fp32 rounding noise).

  host:   w = renorm(top5(softmax(x@gate_W/T)))           [N, E]
          for each expert e: gather its active tokens (count ~5156 of 8192,
          5/8 sparsity) into a padded [C=5248] slab; core e gets expert e's
          weights (bf16) + its gathered tokens (bf16, pre-transposed).
  device: per core: resident bf16 W1/W2/W3 in SBUF (16.8 MB), stream token
          blocks of 512: h1=relu(W1x+b1); h2=relu(W2h1+b2); y=w*(W3h2);
          all matmuls bf16 (1 PE cycle/row — same rate as fp32r but half the
          DMA traffic and no min-free-size penalty). Output f32.
  host:   scatter-add the disjoint (expert, token) outputs into y[N, O],
          plus the (sum_e w)*b3 term.

Device compute: 5248 tok * 512 rows = 2.69M PE rows/core vs 4.19M for the
dense all-expert baseline (1.39 ms measured, PE-bound) -> ~0.9 ms target.
No collectives: expert outputs are disjoint row sets, combined on host.
"""

import numpy as np

import concourse.bass as bass
import concourse.tile as tile
import concourse.mybir as mybir
from concourse import bacc

# Problem constants (hardcoded per contract; kernel.py must be self-contained).
N, D, H, O, E = 8192, 1024, 2048, 1024, 8
CORES = 8
TEMP = float(np.e)
N_ACTIVE = 5
EPS = 1e-8
C_DEFAULT = 5248          # per-expert token capacity (41*128); actual ~5156

F32 = mybir.dt.float32
BF16 = mybir.dt.bfloat16


def build_nc(cap=C_DEFAULT, d=D, h=H, o=O, y_dt=F32, diag="none"):
    """Per-core Bass program: one expert's 3-layer MLP over `cap` tokens."""
    P = 128
    DC = d // P            # 8  contraction chunks, layer 1
    HC = h // P            # 16 h chunks (L1/L2 out, L2/L3 contraction)
    OW = 512
    OT = o // OW           # 2
    TB = 512               # token block
    NQ = cap // P          # 128-token chunks total (41)
    assert cap % P == 0
    blocks = [(b * TB, TB) for b in range(cap // TB)]
    if cap % TB:
        blocks.append((cap - cap % TB, cap % TB))

    nc = bacc.Bacc(None)

    # xg is pre-blocked on host: [block, P, DC, TB] so every block load is
    # per-partition contiguous (128 x 8KB descriptors — the fast DMA shape).
    # The [DC, P, cap] "natural" layout produced 1024 x 1KB strided
    # descriptors per block, which measured SLOWER than a block's compute
    # and made the whole pipeline DMA-bound.
    NBF = cap // TB
    TBT = cap % TB
    xgb_ext = nc.dram_tensor("xgb", [NBF, P, DC, TB], BF16,
                             kind="ExternalInput")
    xgt_ext = (nc.dram_tensor("xgt", [P, DC, TBT], BF16,
                              kind="ExternalInput") if TBT else None)
    w1_ext = nc.dram_tensor("w1p", [P, HC, DC, P], BF16, kind="ExternalInput")
    w2_ext = nc.dram_tensor("w2p", [P, HC, HC, P], BF16, kind="ExternalInput")
    w3_ext = nc.dram_tensor("w3p", [P, OT, HC, OW], BF16, kind="ExternalInput")
    b1_ext = nc.dram_tensor("b1p", [P, HC], F32, kind="ExternalInput")
    b2_ext = nc.dram_tensor("b2p", [P, HC], F32, kind="ExternalInput")
    wg_ext = nc.dram_tensor("wg", [P, NQ], F32, kind="ExternalInput")
    y_ext = nc.dram_tensor("y", [cap, o], y_dt, kind="ExternalOutput")

    with tile.TileContext(nc) as tc:
        with (
            tc.tile_pool(name="const", bufs=1) as cpool,
            tc.tile_pool(name="xgs", bufs=2) as xpool,
            tc.tile_pool(name="acts", bufs=3) as apool,
            tc.tile_pool(name="yout", bufs=4) as ypool,
            tc.tile_pool(name="ps_mm", bufs=4, space="PSUM") as mmps,
            tc.tile_pool(name="ps_out", bufs=4, space="PSUM") as outps,
        ):
            # resident weights. SP queue: w1 first so block-0 L1 starts ~15us
            # in; the bigger w2/w3 stream on the Activation HWDGE queue and
            # are ready before block-0 L2/L3 need them.
            w1_sb = cpool.tile([P, HC, DC, P], BF16, tag="w1")
            nc.sync.dma_start(w1_sb[:], w1_ext[:])
            w2_sb = cpool.tile([P, HC, HC, P], BF16, tag="w2")
            nc.scalar.dma_start(w2_sb[:], w2_ext[:])
            w3_sb = cpool.tile([P, OT, HC, OW], BF16, tag="w3")
            nc.scalar.dma_start(w3_sb[:], w3_ext[:])
            b1_sb = cpool.tile([P, HC], F32, tag="b1")
            nc.gpsimd.dma_start(b1_sb[:], b1_ext[:])
            b2_sb = cpool.tile([P, HC], F32, tag="b2")
            nc.gpsimd.dma_start(b2_sb[:], b2_ext[:])
            wg_sb = cpool.tile([P, NQ], F32, tag="wg")
            nc.gpsimd.dma_start(wg_sb[:], wg_ext[:])

            if diag == "noxdma":
                xg_c = cpool.tile([P, DC, 512], BF16, tag="xgc")
                nc.sync.dma_start(xg_c[:], xgb_ext[0])
            for bi, (t0, tb) in enumerate(blocks):
                if diag == "noxdma":
                    xg_t = xg_c
                else:
                    xg_t = xpool.tile([P, DC, tb], BF16, tag="xg")
                    nc.sync.dma_start(
                        xg_t[:], xgb_ext[bi] if tb == TB else xgt_ext[:])

                # Accumulation chains are interleaved in PAIRS of PSUM banks
                # throughout: back-to-back accumulating matmuls into the SAME
                # bank stall the PE on real HW (write-read turnaround the
                # cost model misses); alternating two banks hides it.

                # layer 1: h1T[j] = relu(sum_dc W1t(j,dc).T @ xgT(dc) + b1)
                h1T = apool.tile([P, HC, tb], BF16, tag="hact")
                for jp in range(0, HC, 2):
                    pss = [mmps.tile([P, tb], F32, tag="mm", name=f"ps1{i}")
                           for i in range(2)]
                    for dc in range(DC):
                        for i in range(2):
                            nc.tensor.matmul(
                                pss[i][:], w1_sb[:, jp + i, dc, :],
                                xg_t[:, dc, 0:tb],
                                start=(dc == 0), stop=(dc == DC - 1),
                            )
                    for i in range(2):
                        nc.scalar.activation(
                            h1T[:, jp + i, :], pss[i][:],
                            mybir.ActivationFunctionType.Relu,
                            bias=b1_sb[:, jp + i:jp + i + 1],
                        )

                # layer 2: h2T[j2] = relu(sum_k W2t(j2,k).T @ h1T(k) + b2)
                h2T = apool.tile([P, HC, tb], BF16, tag="hact")
                for jp in range(0, HC, 2):
                    pss = [mmps.tile([P, tb], F32, tag="mm", name=f"ps2{i}")
                           for i in range(2)]
                    for k in range(HC):
                        for i in range(2):
                            nc.tensor.matmul(
                                pss[i][:], w2_sb[:, jp + i, k, :],
                                h1T[:, k, :],
                                start=(k == 0), stop=(k == HC - 1),
                            )
                    for i in range(2):
                        nc.scalar.activation(
                            h2T[:, jp + i, :], pss[i][:],
                            mybir.ActivationFunctionType.Relu,
                            bias=b2_sb[:, jp + i:jp + i + 1],
                        )

                # layer 3 + routing-weight scale, 128-token chunks in pairs
                nq = tb // P
                for qp in range(0, nq, 2):
                    npair = min(2, nq - qp)
                    for ot in range(OT):
                        psOs = [outps.tile([P, OW], F32, tag="out",
                                           name=f"psO{i}")
                                for i in range(npair)]
                        for k in range(HC):
                            for i in range(npair):
                                qq = qp + i
                                nc.tensor.matmul(
                                    psOs[i][:],
                                    h2T[:, k, qq * P:(qq + 1) * P],
                                    w3_sb[:, ot, k, :],
                                    start=(k == 0), stop=(k == HC - 1),
                                )
                        for i in range(npair):
                            q = t0 // P + qp + i
                            yt = ypool.tile([P, OW], y_dt, tag="yt")
                            nc.vector.tensor_scalar_mul(
                                yt[:], psOs[i][:], wg_sb[:, q:q + 1])
                            nc.gpsimd.dma_start(
                                y_ext[q * P:(q + 1) * P,
                                      ot * OW:(ot + 1) * OW],
                                yt[:])
    nc.compile()
    return nc


# ---------------------------------------------------------------------------
# Host side: routing, packing, PJRT runner (jit once, reusable), unshard.
# ---------------------------------------------------------------------------

def route(x, gate_W, gate_b):
    """fp64 gate; reproduces the reference's fp32 top-5 selection exactly
    (verified: min |p5-p6| gap 1.9e-6 >> fp32 matmul noise ~1e-7)."""
    s = (x.astype(np.float64) @ gate_W.astype(np.float64)
         + gate_b.astype(np.float64)) / TEMP
    s -= s.max(axis=-1, keepdims=True)
    p = np.exp(s)
    p /= p.sum(axis=-1, keepdims=True)
    order = np.argsort(-p, axis=-1, kind="stable")   # jax top_k tie-break
    mask = np.zeros_like(p)
    mask[np.arange(p.shape[0])[:, None], order[:, :N_ACTIVE]] = 1.0
    w = p * mask
    w /= (w.sum(axis=-1, keepdims=True) + EPS)
    return w.astype(np.float32)


def pack_inputs(x, gate_W, gate_b, W1, b1, W2, b2, W3, b3, cap=C_DEFAULT):
    """Route on host; per expert e, gather + pre-tile its tokens/weights."""
    P = 128
    n_experts, d, h = W1.shape
    o = W3.shape[2]
    DC, HC = d // P, h // P
    OW = 512
    OT = o // OW
    NQ = cap // P
    f32 = np.float32

    import ml_dtypes
    bf16 = ml_dtypes.bfloat16

    w = route(x, gate_W, gate_b)                      # [N, E]
    in_maps, aux = [], []
    for e in range(n_experts):
        idx = np.nonzero(w[:, e])[0]
        cnt = len(idx)
        assert cnt <= cap, f"expert {e}: {cnt} active tokens > cap {cap}"
        xg = np.zeros((cap, d), f32)
        xg[:cnt] = x[idx]
        # pre-blocked layouts (see build_nc): xgb[b,p,dc,t] = xg[b*TB+t, dc*128+p]
        TB = 512
        NBF, TBT = cap // TB, cap % TB
        v = xg.reshape(cap, DC, P)
        xgb = np.ascontiguousarray(
            v[:NBF * TB].reshape(NBF, TB, DC, P).transpose(0, 3, 2, 1)
        ).astype(bf16)
        xgt = (np.ascontiguousarray(
            v[NBF * TB:].transpose(2, 1, 0)).astype(bf16) if TBT else None)
        wgv = np.zeros((cap,), f32)
        wgv[:cnt] = w[idx, e]
        wgp = np.ascontiguousarray(wgv.reshape(NQ, P).T)      # [P, NQ]
        # (p, j, dc, ph) = W1[e, dc*128+p, j*128+ph]
        w1p = np.ascontiguousarray(
            W1[e].reshape(DC, P, HC, P).transpose(1, 2, 0, 3)).astype(bf16)
        # (p, j2, k, ph2) = W2[e, k*128+p, j2*128+ph2]
        w2p = np.ascontiguousarray(
            W2[e].reshape(HC, P, HC, P).transpose(1, 2, 0, 3)).astype(bf16)
        # (p, ot, k, ow) = W3[e, k*128+p, ot*512+ow]
        w3p = np.ascontiguousarray(
            W3[e].reshape(HC, P, OT, OW).transpose(1, 2, 0, 3)).astype(bf16)
        b1p = np.ascontiguousarray(b1[e].reshape(HC, P).T).astype(f32)
        b2p = np.ascontiguousarray(b2[e].reshape(HC, P).T).astype(f32)
        m = dict(xgb=xgb, w1p=w1p, w2p=w2p, w3p=w3p,
                 b1p=b1p, b2p=b2p, wg=wgp)
        if xgt is not None:
            m["xgt"] = xgt
        in_maps.append(m)
        aux.append((idx, cnt))
    return in_maps, aux, w


def unshard(res, aux, w, b3, o=O):
    """Scatter-add disjoint per-expert outputs; add the w @ b3 term."""
    y = np.zeros((N, o), np.float32)
    for e, (idx, cnt) in enumerate(aux):
        y[idx] += res[e]["y"][:cnt]
    y += w @ b3.astype(np.float32)                    # b3 is [E, O]
    return y


class SpmdRunner:
    """jit-once PJRT SPMD runner (mirrors bass2jax.run_bass_via_pjrt but
    reusable across calls so the NEFF compile is paid once)."""

    def __init__(self, nc, n_cores):
        import jax
        from jax.sharding import Mesh, PartitionSpec
        from jax.experimental.shard_map import shard_map
        from concourse import bass2jax as b2j

        b2j.install_neuronx_cc_hook()
        self.nc = nc
        self.n_cores = n_cores
        in_names, out_names, out_avals, zero_outs = [], [], [], []
        for alloc in nc.m.functions[0].allocations:
            if not isinstance(alloc, mybir.MemoryLocationSet):
                continue
            name = alloc.memorylocations[0].name
            if alloc.kind == "ExternalInput":
                if not (nc.partition_id_tensor
                        and name == nc.partition_id_tensor.name):
                    in_names.append(name)
            elif alloc.kind == "ExternalOutput":
                out_names.append(name)
                shape = tuple(alloc.tensor_shape)
                dtype = mybir.dt.np(alloc.dtype)
                out_avals.append(jax.core.ShapedArray(shape, dtype))
                zero_outs.append(np.zeros(shape, dtype))
        self.in_names, self.out_names = in_names, out_names
        self.out_avals, self.zero_outs = out_avals, zero_outs
        n_params, n_outs = len(in_names), len(out_names)
        self.n_params = n_params
        all_in_names = list(in_names) + list(out_names)
        partition_name = (nc.partition_id_tensor.name
                          if nc.partition_id_tensor else None)
        if partition_name is not None:
            all_in_names.append(partition_name)

        def _body(*args):
            operands = list(args)
            if partition_name is not None:
                operands.append(b2j.partition_id_tensor())
            outs = b2j._bass_exec_p.bind(
                *operands,
                out_avals=tuple(out_avals),
                in_names=tuple(all_in_names),
                out_names=tuple(out_names),
                lowering_input_output_aliases=(),
                sim_require_finite=True,
                sim_require_nnan=True,
                nc=nc,
            )
            return tuple(outs)

        devices = jax.devices()[:n_cores]
        self.mesh = Mesh(np.asarray(devices), ("core",))
        in_specs = (PartitionSpec("core"),) * (n_params + n_outs)
        out_specs = (PartitionSpec("core"),) * n_outs
        donate = tuple(range(n_params, n_params + n_outs))
        self.fn = jax.jit(
            shard_map(_body, mesh=self.mesh, in_specs=in_specs,
                      out_specs=out_specs, check_rep=False),
            donate_argnums=donate, keep_unused=True,
        )
        self.jax = jax

    def prep(self, in_maps):
        concat = [
            np.concatenate([np.asarray(m[n]) for m in in_maps], axis=0)
            for n in self.in_names
        ]
        return concat

    def zeros(self):
        return [np.zeros((self.n_cores * z.shape[0], *z.shape[1:]), z.dtype)
                for z in self.zero_outs]

    def __call__(self, concat_in, concat_zeros):
        out = self.fn(*concat_in, *concat_zeros)
        self.jax.block_until_ready(out)
        return out

    def time_pipelined(self, concat_in, k=33, reps=2):
        """Dispatch k executions back-to-back (async), block once: the device
        serializes them, so (T_k - T_1)/(k-1) ~= per-execution device time
        with dispatch overhead amortized."""
        import time as _time
        import jax
        from jax.sharding import NamedSharding, PartitionSpec
        sh = NamedSharding(self.mesh, PartitionSpec("core"))
        darrs = [jax.device_put(a, sh) for a in concat_in]
        jax.block_until_ready(darrs)

        def run_batch(n):
            zs = [[jax.device_put(z, sh) for z in self.zeros()]
                  for _ in range(n)]
            for z in zs:
                jax.block_until_ready(z)
            t0 = _time.perf_counter()
            outs = [self.fn(*darrs, *zs[i]) for i in range(n)]
            jax.block_until_ready(outs)
            return _time.perf_counter() - t0

        run_batch(2)  # warm
        t1 = min(run_batch(1) for _ in range(3))
        tk = min(run_batch(k) for _ in range(reps))
        per = (tk - t1) / (k - 1)
        return per, t1, tk

    def split_outs(self, out_arrs):
        res = []
        for c in range(self.n_cores):
            res.append({
                name: np.asarray(out_arrs[i]).reshape(
                    self.n_cores, *self.out_avals[i].shape)[c]
                for i, name in enumerate(self.out_names)
            })
        return res


_CACHE = {}


def _get_runner(cap=C_DEFAULT):
    if cap not in _CACHE:
        nc = build_nc(cap=cap)
        _CACHE[cap] = SpmdRunner(nc, CORES)
    return _CACHE[cap]


def kernel(**inputs):
    x = np.asarray(inputs["x"], dtype=np.float32)
    gate_W = np.asarray(inputs["gate_W"], dtype=np.float32)
    gate_b = np.asarray(inputs["gate_b"], dtype=np.float32)
    W1 = np.asarray(inputs["W1"], dtype=np.float32)
    b1 = np.asarray(inputs["b1"], dtype=np.float32)
    W2 = np.asarray(inputs["W2"], dtype=np.float32)
    b2 = np.asarray(inputs["b2"], dtype=np.float32)
    W3 = np.asarray(inputs["W3"], dtype=np.float32)
    b3 = np.asarray(inputs["b3"], dtype=np.float32)

    # capacity: fixed 5248 covers the reference data (max 5156); if some
    # other input needs more, rebuild at the next 128-multiple.
    w = route(x, gate_W, gate_b)
    maxcnt = int((w > 0).sum(axis=0).max())
    cap = C_DEFAULT if maxcnt <= C_DEFAULT else ((maxcnt + 127) // 128) * 128
    runner = _get_runner(cap)
    in_maps, aux, w = pack_inputs(x, gate_W, gate_b, W1, b1, W2, b2, W3, b3,
                                  cap=cap)
    out = runner(runner.prep(in_maps), runner.zeros())
    res = runner.split_outs(out)
    return unshard(res, aux, w, b3).astype(np.float32)


if __name__ == "__main__":
    print("building...")
    nc = build_nc()
    print("built ok")



# revision 2
# speedup vs baseline: 1.4252x; 1.4252x over previous
"""Trainium2 Bass kernel for nn_MoE_25005299597538 (moe_routing).

Strategy: expert-parallel with host-side routing (the gate is 0.01% of the
FLOPs; the reference's fp32 top-5 selection is reproduced exactly by an fp64
host gate — verified min p5/p6 gap 1.9e-6 >> fp32 rounding noise).

  host:   w = renorm(top5(softmax(x@gate_W/T)))           [N, E]
          for each expert e: gather its active tokens (count ~5156 of 8192,
          5/8 sparsity) into a padded [C=5248] slab; core e gets expert e's
          weights (bf16) + its gathered tokens (bf16, pre-transposed).
  device: per core: resident bf16 W1/W2/W3 in SBUF (16.8 MB), stream token
          blocks of 512: h1=relu(W1x+b1); h2=relu(W2h1+b2); y=w*(W3h2);
          all matmuls bf16 (1 PE cycle/row). Output bf16 (halves the
          21.5MB y DMA; the host combine upcasts to f32).
  host:   scatter-add the disjoint (expert, token) outputs into y[N, O],
          plus the (sum_e w)*b3 term.

DMA schedule (v2): measured ~272 GB/s per HWDGE queue on this device and
a ~285us/exec cost attributable to the 16.8MB weight preload, so the
preload is spread arrival-ordered across all 3 queues:
  sync (SP):       xgb[0], w1 (j-halves), xgb[2,4,...]
  scalar (ACT):    w2 k-lower-half, w3[ot=0], xgb[1,3,...]
  gpsimd (SWDGE):  b1, b2, wg, w2 k-upper-half, w3[ot=1], y-out tiles
w2 is host-packed k-half-major ("w2q") so each queue's chunk is a
per-partition-contiguous transfer.

Device compute: 5248 tok * 512 PE-rows = 2.69M rows/core ~= 1.12ms floor at
the measured 2.46GHz PE clock; per-MM overhead ~20ns (LDWEIGHTS are hidden
by the PE reorder window — verified by IR-surgery A/B). No collectives:
expert outputs are disjoint row sets, combined on host.
"""

import numpy as np

import concourse.bass as bass
import concourse.tile as tile
import concourse.mybir as mybir
from concourse import bacc

# Problem constants (hardcoded per contract; kernel.py must be self-contained).
N, D, H, O, E = 8192, 1024, 2048, 1024, 8
CORES = 8
TEMP = float(np.e)
N_ACTIVE = 5
EPS = 1e-8
C_DEFAULT = 5248          # per-expert token capacity (41*128); actual ~5156

F32 = mybir.dt.float32
BF16 = mybir.dt.bfloat16


def build_nc(cap=C_DEFAULT, d=D, h=H, o=O, y_dt=BF16):
    """Per-core Bass program: one expert's 3-layer MLP over `cap` tokens."""
    P = 128
    DC = d // P            # 8  contraction chunks, layer 1
    HC = h // P            # 16 h chunks (L1/L2 out, L2/L3 contraction)
    OW = 512
    OT = o // OW           # 2
    TB = 512               # token block
    NQ = cap // P
    assert cap % P == 0
    blocks = [(b * TB, TB) for b in range(cap // TB)]
    if cap % TB:
        blocks.append((cap - cap % TB, cap % TB))

    nc = bacc.Bacc(None)

    # xg is pre-blocked on host: [block, P, DC, TB] so every block load is
    # per-partition contiguous (128 x 8KB descriptors — the fast DMA shape).
    NBF = cap // TB
    TBT = cap % TB
    xgb_ext = nc.dram_tensor("xgb", [NBF, P, DC, TB], BF16,
                             kind="ExternalInput")
    xgt_ext = (nc.dram_tensor("xgt", [P, DC, TBT], BF16,
                              kind="ExternalInput") if TBT else None)
    w1_ext = nc.dram_tensor("w1p", [P, HC, DC, P], BF16, kind="ExternalInput")
    # w2 packed k-half-major so each half is per-partition contiguous:
    # w2q[p, h, j, kk, q] = W2[(h*8+kk)*128+p, j*128+q]
    w2_ext = nc.dram_tensor("w2q", [P, 2, HC, HC // 2, P], BF16,
                            kind="ExternalInput")
    w3_ext = nc.dram_tensor("w3p", [P, OT, HC, OW], BF16, kind="ExternalInput")
    b1_ext = nc.dram_tensor("b1p", [P, HC], F32, kind="ExternalInput")
    b2_ext = nc.dram_tensor("b2p", [P, HC], F32, kind="ExternalInput")
    wg_ext = nc.dram_tensor("wg", [P, NQ], F32, kind="ExternalInput")
    y_ext = nc.dram_tensor("y", [cap, o], y_dt, kind="ExternalOutput")

    HCH = HC // 2          # 8: w1 j-split and w2 k-split point

    with tile.TileContext(nc) as tc:
        with (
            tc.tile_pool(name="const", bufs=1) as cpool,
            tc.tile_pool(name="xgs", bufs=2) as xpool,
            tc.tile_pool(name="acts", bufs=3) as apool,
            tc.tile_pool(name="yout", bufs=4) as ypool,
            tc.tile_pool(name="ps_mm", bufs=4, space="PSUM") as mmps,
            tc.tile_pool(name="ps_out", bufs=4, space="PSUM") as outps,
        ):
            w1a = cpool.tile([P, HCH, DC, P], BF16, tag="w1a")
            w1b = cpool.tile([P, HCH, DC, P], BF16, tag="w1b")
            w2a = cpool.tile([P, HC, HCH, P], BF16, tag="w2a")
            w2b = cpool.tile([P, HC, HCH, P], BF16, tag="w2b")
            w3a = cpool.tile([P, HC, OW], BF16, tag="w3a")
            w3b = cpool.tile([P, HC, OW], BF16, tag="w3b")
            b1_sb = cpool.tile([P, HC], F32, tag="b1")
            nc.gpsimd.dma_start(b1_sb[:], b1_ext[:])
            b2_sb = cpool.tile([P, HC], F32, tag="b2")
            nc.gpsimd.dma_start(b2_sb[:], b2_ext[:])
            wg_sb = cpool.tile([P, NQ], F32, tag="wg")
            nc.gpsimd.dma_start(wg_sb[:], wg_ext[:])
            # weight halves split across the scalar + gpsimd queues,
            # k-half of w2 first (L2 consumes k ascending)
            nc.scalar.dma_start(w2a[:], w2_ext[:, 0])
            nc.gpsimd.dma_start(w2b[:], w2_ext[:, 1])
            nc.scalar.dma_start(w3a[:], w3_ext[:, 0])
            nc.gpsimd.dma_start(w3b[:], w3_ext[:, 1])

            first_w1 = [True]

            def w1_sb(j):
                return w1a[:, j, :, :] if j < HCH else w1b[:, j - HCH, :, :]

            def w2_sb(j, k):
                return (w2a[:, j, k, :] if k < HCH
                        else w2b[:, j, k - HCH, :])

            def w3_sb(ot, k):
                return w3a[:, k, :] if ot == 0 else w3b[:, k, :]

            for bi, (t0, tb) in enumerate(blocks):
                xg_t = xpool.tile([P, DC, tb], BF16, tag="xg")
                q = nc.sync if bi % 2 == 0 else nc.scalar
                q.dma_start(
                    xg_t[:], xgb_ext[bi] if tb == TB else xgt_ext[:])
                if first_w1[0]:
                    # w1 rides sync AFTER block-0 x so L1 can start early
                    nc.sync.dma_start(w1a[:], w1_ext[:, 0:HCH, :, :])
                    nc.sync.dma_start(w1b[:], w1_ext[:, HCH:HC, :, :])
                    first_w1[0] = False

                # Accumulation chains are interleaved in PAIRS of PSUM banks
                # throughout: back-to-back accumulating matmuls into the SAME
                # bank stall the PE (write-read turnaround); alternating two
                # banks hides it.

                # layer 1: h1T[j] = relu(sum_dc W1t(j,dc).T @ xgT(dc) + b1)
                h1T = apool.tile([P, HC, tb], BF16, tag="hact")
                for jp in range(0, HC, 2):
                    pss = [mmps.tile([P, tb], F32, tag="mm", name=f"ps1{i}")
                           for i in range(2)]
                    for dc in range(DC):
                        for i in range(2):
                            nc.tensor.matmul(
                                pss[i][:], w1_sb(jp + i)[:, dc, :],
                                xg_t[:, dc, 0:tb],
                                start=(dc == 0), stop=(dc == DC - 1),
                            )
                    for i in range(2):
                        nc.scalar.activation(
                            h1T[:, jp + i, :], pss[i][:],
                            mybir.ActivationFunctionType.Relu,
                            bias=b1_sb[:, jp + i:jp + i + 1],
                        )

                # layer 2: h2T[j2] = relu(sum_k W2t(j2,k).T @ h1T(k) + b2)
                h2T = apool.tile([P, HC, tb], BF16, tag="hact")
                for jp in range(0, HC, 2):
                    pss = [mmps.tile([P, tb], F32, tag="mm", name=f"ps2{i}")
                           for i in range(2)]
                    for k in range(HC):
                        for i in range(2):
                            nc.tensor.matmul(
                                pss[i][:], w2_sb(jp + i, k),
                                h1T[:, k, :],
                                start=(k == 0), stop=(k == HC - 1),
                            )
                    for i in range(2):
                        nc.scalar.activation(
                            h2T[:, jp + i, :], pss[i][:],
                            mybir.ActivationFunctionType.Relu,
                            bias=b2_sb[:, jp + i:jp + i + 1],
                        )

                # layer 3 + routing-weight scale, 128-token chunks in pairs
                nq = tb // P
                for qp in range(0, nq, 2):
                    npair = min(2, nq - qp)
                    for ot in range(OT):
                        psOs = [outps.tile([P, OW], F32, tag="out",
                                           name=f"psO{i}")
                                for i in range(npair)]
                        for k in range(HC):
                            for i in range(npair):
                                qq = qp + i
                                nc.tensor.matmul(
                                    psOs[i][:],
                                    h2T[:, k, qq * P:(qq + 1) * P],
                                    w3_sb(ot, k),
                                    start=(k == 0), stop=(k == HC - 1),
                                )
                        for i in range(npair):
                            q2 = t0 // P + qp + i
                            yt = ypool.tile([P, OW], y_dt, tag="yt")
                            nc.vector.tensor_scalar_mul(
                                yt[:], psOs[i][:], wg_sb[:, q2:q2 + 1])
                            nc.gpsimd.dma_start(
                                y_ext[q2 * P:(q2 + 1) * P,
                                      ot * OW:(ot + 1) * OW],
                                yt[:])
    nc.compile()
    return nc


# ---------------------------------------------------------------------------
# Host side: routing, packing, PJRT runner (jit once, reusable), unshard.
# ---------------------------------------------------------------------------

def route(x, gate_W, gate_b):
    """fp64 gate; reproduces the reference's fp32 top-5 selection exactly
    (verified: min |p5-p6| gap 1.9e-6 >> fp32 matmul noise ~1e-7)."""
    s = (x.astype(np.float64) @ gate_W.astype(np.float64)
         + gate_b.astype(np.float64)) / TEMP
    s -= s.max(axis=-1, keepdims=True)
    p = np.exp(s)
    p /= p.sum(axis=-1, keepdims=True)
    order = np.argsort(-p, axis=-1, kind="stable")   # jax top_k tie-break
    mask = np.zeros_like(p)
    mask[np.arange(p.shape[0])[:, None], order[:, :N_ACTIVE]] = 1.0
    w = p * mask
    w /= (w.sum(axis=-1, keepdims=True) + EPS)
    return w.astype(np.float32)


def pack_inputs(x, gate_W, gate_b, W1, b1, W2, b2, W3, b3, cap=C_DEFAULT):
    """Route on host; per expert e, gather + pre-tile its tokens/weights."""
    P = 128
    n_experts, d, h = W1.shape
    o = W3.shape[2]
    DC, HC = d // P, h // P
    OW = 512
    OT = o // OW
    NQ = cap // P
    f32 = np.float32

    import ml_dtypes
    bf16 = ml_dtypes.bfloat16

    w = route(x, gate_W, gate_b)                      # [N, E]
    in_maps, aux = [], []
    for e in range(n_experts):
        idx = np.nonzero(w[:, e])[0]
        cnt = len(idx)
        assert cnt <= cap, f"expert {e}: {cnt} active tokens > cap {cap}"
        xg = np.zeros((cap, d), f32)
        xg[:cnt] = x[idx]
        # pre-blocked layouts (see build_nc): xgb[b,p,dc,t] = xg[b*TB+t, dc*128+p]
        TB = 512
        NBF, TBT = cap // TB, cap % TB
        v = xg.reshape(cap, DC, P)
        xgb = np.ascontiguousarray(
            v[:NBF * TB].reshape(NBF, TB, DC, P).transpose(0, 3, 2, 1)
        ).astype(bf16)
        xgt = (np.ascontiguousarray(
            v[NBF * TB:].transpose(2, 1, 0)).astype(bf16) if TBT else None)
        wgv = np.zeros((cap,), f32)
        wgv[:cnt] = w[idx, e]
        wgp = np.ascontiguousarray(wgv.reshape(NQ, P).T)      # [P, NQ]
        # (p, j, dc, ph) = W1[e, dc*128+p, j*128+ph]
        w1p = np.ascontiguousarray(
            W1[e].reshape(DC, P, HC, P).transpose(1, 2, 0, 3)).astype(bf16)
        # w2q[p, half, j, kk, q] = W2[e, (half*8+kk)*128+p, j*128+q]
        w2q = np.ascontiguousarray(
            W2[e].reshape(2, HC // 2, P, HC, P)    # (half, kk, p, j, q)
            .transpose(2, 0, 3, 1, 4)).astype(bf16)  # (p, half, j, kk, q)
        # (p, ot, k, ow) = W3[e, k*128+p, ot*512+ow]
        w3p = np.ascontiguousarray(
            W3[e].reshape(HC, P, OT, OW).transpose(1, 2, 0, 3)).astype(bf16)
        b1p = np.ascontiguousarray(b1[e].reshape(HC, P).T).astype(f32)
        b2p = np.ascontiguousarray(b2[e].reshape(HC, P).T).astype(f32)
        m = dict(xgb=xgb, w1p=w1p, w2q=w2q, w3p=w3p,
                 b1p=b1p, b2p=b2p, wg=wgp)
        if xgt is not None:
            m["xgt"] = xgt
        in_maps.append(m)
        aux.append((idx, cnt))
    return in_maps, aux, w


def unshard(res, aux, w, b3, o=O):
    """Scatter-add disjoint per-expert outputs; add the w @ b3 term."""
    y = np.zeros((N, o), np.float32)
    for e, (idx, cnt) in enumerate(aux):
        y[idx] += res[e]["y"][:cnt].astype(np.float32)
    y += w @ b3.astype(np.float32)                    # b3 is [E, O]
    return y


class SpmdRunner:
    """jit-once PJRT SPMD runner (mirrors bass2jax.run_bass_via_pjrt but
    reusable across calls so the NEFF compile is paid once)."""

    def __init__(self, nc, n_cores):
        import jax
        from jax.sharding import Mesh, PartitionSpec
        from jax.experimental.shard_map import shard_map
        from concourse import bass2jax as b2j

        b2j.install_neuronx_cc_hook()
        self.nc = nc
        self.n_cores = n_cores
        in_names, out_names, out_avals, zero_outs = [], [], [], []
        for alloc in nc.m.functions[0].allocations:
            if not isinstance(alloc, mybir.MemoryLocationSet):
                continue
            name = alloc.memorylocations[0].name
            if alloc.kind == "ExternalInput":
                if not (nc.partition_id_tensor
                        and name == nc.partition_id_tensor.name):
                    in_names.append(name)
            elif alloc.kind == "ExternalOutput":
                out_names.append(name)
                shape = tuple(alloc.tensor_shape)
                dtype = mybir.dt.np(alloc.dtype)
                out_avals.append(jax.core.ShapedArray(shape, dtype))
                zero_outs.append(np.zeros(shape, dtype))
        self.in_names, self.out_names = in_names, out_names
        self.out_avals, self.zero_outs = out_avals, zero_outs
        n_params, n_outs = len(in_names), len(out_names)
        self.n_params = n_params
        all_in_names = list(in_names) + list(out_names)
        partition_name = (nc.partition_id_tensor.name
                          if nc.partition_id_tensor else None)
        if partition_name is not None:
            all_in_names.append(partition_name)

        def _body(*args):
            operands = list(args)
            if partition_name is not None:
                operands.append(b2j.partition_id_tensor())
            outs = b2j._bass_exec_p.bind(
                *operands,
                out_avals=tuple(out_avals),
                in_names=tuple(all_in_names),
                out_names=tuple(out_names),
                lowering_input_output_aliases=(),
                sim_require_finite=True,
                sim_require_nnan=True,
                nc=nc,
            )
            return tuple(outs)

        devices = jax.devices()[:n_cores]
        self.mesh = Mesh(np.asarray(devices), ("core",))
        in_specs = (PartitionSpec("core"),) * (n_params + n_outs)
        out_specs = (PartitionSpec("core"),) * n_outs
        donate = tuple(range(n_params, n_params + n_outs))
        self.fn = jax.jit(
            shard_map(_body, mesh=self.mesh, in_specs=in_specs,
                      out_specs=out_specs, check_rep=False),
            donate_argnums=donate, keep_unused=True,
        )
        # non-donating variant for repeated timing (zeros uploaded once,
        # outputs allocated fresh each exec -> every exec is observable)
        self.fn_nodonate = jax.jit(
            shard_map(_body, mesh=self.mesh, in_specs=in_specs,
                      out_specs=out_specs, check_rep=False),
            keep_unused=True,
        )
        self.jax = jax

    def prep(self, in_maps):
        concat = [
            np.concatenate([np.asarray(m[n]) for m in in_maps], axis=0)
            for n in self.in_names
        ]
        return concat

    def zeros(self):
        return [np.zeros((self.n_cores * z.shape[0], *z.shape[1:]), z.dtype)
                for z in self.zero_outs]

    def __call__(self, concat_in, concat_zeros):
        out = self.fn(*concat_in, *concat_zeros)
        self.jax.block_until_ready(out)
        return out

    def time_batches(self, concat_in, rounds=8, small=1, big=33):
        """Per-exec device time via differential batches on the
        non-donating fn: per = (min T_big - min T_small)/(big - small).
        Each timed batch is preceded by an untimed absorber exec (soaks
        executable-switch and queue-warm costs)."""
        import time as _time
        import jax
        from jax.sharding import NamedSharding, PartitionSpec
        sh = NamedSharding(self.mesh, PartitionSpec("core"))
        darrs = [jax.device_put(a, sh) for a in concat_in]
        jax.block_until_ready(darrs)
        zeros = [jax.device_put(z, sh) for z in self.zeros()]
        jax.block_until_ready(zeros)
        o = self.fn_nodonate(*darrs, *zeros)
        jax.block_until_ready(o)
        del o

        def batch(k):
            o = self.fn_nodonate(*darrs, *zeros)     # absorber
            jax.block_until_ready(o)
            del o
            outs = []
            t0 = _time.perf_counter()
            for _ in range(k):
                outs.append(self.fn_nodonate(*darrs, *zeros))
            jax.block_until_ready(outs)
            dt = _time.perf_counter() - t0
            del outs
            return dt

        ts, tb = [], []
        for _ in range(rounds):
            ts.append(batch(small))
            tb.append(batch(big))
        per = (min(tb) - min(ts)) / (big - small)
        return per, ts, tb

    def split_outs(self, out_arrs):
        res = []
        for c in range(self.n_cores):
            res.append({
                name: np.asarray(out_arrs[i]).reshape(
                    self.n_cores, *self.out_avals[i].shape)[c]
                for i, name in enumerate(self.out_names)
            })
        return res


_CACHE = {}


def _get_runner(cap=C_DEFAULT):
    if cap not in _CACHE:
        nc = build_nc(cap=cap)
        _CACHE[cap] = SpmdRunner(nc, CORES)
    return _CACHE[cap]


def kernel(**inputs):
    x = np.asarray(inputs["x"], dtype=np.float32)
    gate_W = np.asarray(inputs["gate_W"], dtype=np.float32)
    gate_b = np.asarray(inputs["gate_b"], dtype=np.float32)
    W1 = np.asarray(inputs["W1"], dtype=np.float32)
    b1 = np.asarray(inputs["b1"], dtype=np.float32)
    W2 = np.asarray(inputs["W2"], dtype=np.float32)
    b2 = np.asarray(inputs["b2"], dtype=np.float32)
    W3 = np.asarray(inputs["W3"], dtype=np.float32)
    b3 = np.asarray(inputs["b3"], dtype=np.float32)

    # capacity: fixed 5248 covers the reference data (max 5156); if some
    # other input needs more, rebuild at the next 128-multiple.
    w = route(x, gate_W, gate_b)
    maxcnt = int((w > 0).sum(axis=0).max())
    cap = C_DEFAULT if maxcnt <= C_DEFAULT else ((maxcnt + 127) // 128) * 128
    runner = _get_runner(cap)
    in_maps, aux, w = pack_inputs(x, gate_W, gate_b, W1, b1, W2, b2, W3, b3,
                                  cap=cap)
    out = runner(runner.prep(in_maps), runner.zeros())
    res = runner.split_outs(out)
    return unshard(res, aux, w, b3).astype(np.float32)


if __name__ == "__main__":
    print("building...")
    nc = build_nc()
    print("built ok")


# revision 5
# speedup vs baseline: 1.4403x; 1.0106x over previous
"""Trainium2 Bass kernel for nn_MoE_25005299597538 (moe_routing).

Strategy: expert-parallel with host-side routing (the gate is 0.01% of the
FLOPs; the reference's fp32 top-5 selection is reproduced exactly by an fp64
host gate — verified min p5/p6 gap 1.9e-6 >> fp32 rounding noise).

  host:   w = renorm(top5(softmax(x@gate_W/T)))           [N, E]
          for each expert e: gather its active tokens (count ~5156 of 8192,
          5/8 sparsity) into a padded [C=5248] slab; core e gets expert e's
          weights (bf16) + its gathered tokens (bf16, pre-transposed).
  device: per core: resident bf16 W1/W2/W3 in SBUF (16.8 MB), stream token
          blocks of 512: h1=relu(W1x+b1); h2=relu(W2h1+b2); y=w*(W3h2);
          all matmuls bf16 (1 PE cycle/row). Output bf16 (halves the
          21.5MB y DMA; the host combine upcasts to f32).
  host:   scatter-add the disjoint (expert, token) outputs into y[N, O],
          plus the (sum_e w)*b3 term.

DMA schedule (v3): measured ~272 GB/s per HWDGE queue on this device and
a ~285us/exec cost attributable to the 16.8MB weight preload. All big
weights ride the scalar HWDGE queue in consumption order (12.6MB = 46us,
each chunk arrives before its first use); gpsimd's SWDGE carries only the
small y-out tiles + biases, because SWDGE descriptor generation shares an
SBUF port with DVE 2-port ops (our y-scale tensor_scalar) and big weight
transfers there measured slower:
  sync (SP):       xgb[0], w1 (j-halves), xgb[2,4,...]
  scalar (ACT):    w2 (k-halves), w3 (ot-halves), xgb[1,3,...]
  gpsimd (SWDGE):  b1, b2, wg, y-out tiles
w2 is host-packed k-half-major ("w2q") so each chunk is a
per-partition-contiguous transfer.

Device compute: 5248 tok * 512 PE-rows = 2.69M rows/core ~= 1.12ms floor at
the measured 2.46GHz PE clock; per-MM overhead ~20ns (LDWEIGHTS are hidden
by the PE reorder window — verified by IR-surgery A/B). No collectives:
expert outputs are disjoint row sets, combined on host.
"""

import numpy as np

import concourse.bass as bass
import concourse.tile as tile
import concourse.mybir as mybir
from concourse import bacc

# Problem constants (hardcoded per contract; kernel.py must be self-contained).
N, D, H, O, E = 8192, 1024, 2048, 1024, 8
CORES = 8
TEMP = float(np.e)
N_ACTIVE = 5
EPS = 1e-8
C_DEFAULT = 5248          # per-expert token capacity (41*128); actual ~5156

F32 = mybir.dt.float32
BF16 = mybir.dt.bfloat16


def build_nc(cap=C_DEFAULT, d=D, h=H, o=O, y_dt=BF16, wq_mode="scalar"):
    """Per-core Bass program: one expert's 3-layer MLP over `cap` tokens."""
    P = 128
    DC = d // P            # 8  contraction chunks, layer 1
    HC = h // P            # 16 h chunks (L1/L2 out, L2/L3 contraction)
    OW = 512
    OT = o // OW           # 2
    TB = 512               # token block
    NQ = cap // P
    assert cap % P == 0
    blocks = [(b * TB, TB) for b in range(cap // TB)]
    if cap % TB:
        blocks.append((cap - cap % TB, cap % TB))

    nc = bacc.Bacc(None)

    # xg is pre-blocked on host: [block, P, DC, TB] so every block load is
    # per-partition contiguous (128 x 8KB descriptors — the fast DMA shape).
    NBF = cap // TB
    TBT = cap % TB
    xgb_ext = nc.dram_tensor("xgb", [NBF, P, DC, TB], BF16,
                             kind="ExternalInput")
    xgt_ext = (nc.dram_tensor("xgt", [P, DC, TBT], BF16,
                              kind="ExternalInput") if TBT else None)
    w1_ext = nc.dram_tensor("w1p", [P, HC, DC, P], BF16, kind="ExternalInput")
    # w2 packed k-half-major so each half is per-partition contiguous:
    # w2q[p, h, j, kk, q] = W2[(h*8+kk)*128+p, j*128+q]
    w2_ext = nc.dram_tensor("w2q", [P, 2, HC, HC // 2, P], BF16,
                            kind="ExternalInput")
    w3_ext = nc.dram_tensor("w3p", [P, OT, HC, OW], BF16, kind="ExternalInput")
    b1_ext = nc.dram_tensor("b1p", [P, HC], F32, kind="ExternalInput")
    b2_ext = nc.dram_tensor("b2p", [P, HC], F32, kind="ExternalInput")
    wg_ext = nc.dram_tensor("wg", [P, NQ], F32, kind="ExternalInput")
    y_ext = nc.dram_tensor("y", [cap, o], y_dt, kind="ExternalOutput")

    HCH = HC // 2          # 8: w1 j-split and w2 k-split point

    with tile.TileContext(nc) as tc:
        with (
            tc.tile_pool(name="const", bufs=1) as cpool,
            tc.tile_pool(name="xgs", bufs=2) as xpool,
            tc.tile_pool(name="acts", bufs=3) as apool,
            tc.tile_pool(name="yout", bufs=4) as ypool,
            tc.tile_pool(name="ps_mm", bufs=4, space="PSUM") as mmps,
            tc.tile_pool(name="ps_out", bufs=4, space="PSUM") as outps,
        ):
            w1a = cpool.tile([P, HCH, DC, P], BF16, tag="w1a")
            w1b = cpool.tile([P, HCH, DC, P], BF16, tag="w1b")
            w2a = cpool.tile([P, HC, HCH, P], BF16, tag="w2a")
            w2b = cpool.tile([P, HC, HCH, P], BF16, tag="w2b")
            w3a = cpool.tile([P, HC, OW], BF16, tag="w3a")
            w3b = cpool.tile([P, HC, OW], BF16, tag="w3b")
            b1_sb = cpool.tile([P, HC], F32, tag="b1")
            nc.gpsimd.dma_start(b1_sb[:], b1_ext[:])
            b2_sb = cpool.tile([P, HC], F32, tag="b2")
            nc.gpsimd.dma_start(b2_sb[:], b2_ext[:])
            wg_sb = cpool.tile([P, NQ], F32, tag="wg")
            nc.gpsimd.dma_start(wg_sb[:], wg_ext[:])
            # w2/w3 queue placement: "split3" uses scalar+gpsimd; "scalar"
            # keeps all big weights on the scalar HWDGE queue so gpsimd's
            # SWDGE (whose descriptor generation shares an SBUF port with
            # DVE 2-port ops) only carries the small y-out tiles.
            if wq_mode == "split3":
                nc.scalar.dma_start(w2a[:], w2_ext[:, 0])
                nc.gpsimd.dma_start(w2b[:], w2_ext[:, 1])
                nc.scalar.dma_start(w3a[:], w3_ext[:, 0])
                nc.gpsimd.dma_start(w3b[:], w3_ext[:, 1])
            elif wq_mode == "scalar":
                nc.scalar.dma_start(w2a[:], w2_ext[:, 0])
                nc.scalar.dma_start(w2b[:], w2_ext[:, 1])
                nc.scalar.dma_start(w3a[:], w3_ext[:, 0])
                nc.scalar.dma_start(w3b[:], w3_ext[:, 1])
            elif wq_mode == "scalar_chunked":
                HQ = 4
                for jq in range(0, HC, HQ):
                    nc.scalar.dma_start(w2a[:, jq:jq + HQ],
                                        w2_ext[:, 0, jq:jq + HQ])
                    nc.scalar.dma_start(w2b[:, jq:jq + HQ],
                                        w2_ext[:, 1, jq:jq + HQ])
                nc.scalar.dma_start(w3a[:], w3_ext[:, 0])
                nc.scalar.dma_start(w3b[:], w3_ext[:, 1])
            else:
                raise ValueError(wq_mode)

            first_w1 = [True]

            def w1_sb(j):
                return w1a[:, j, :, :] if j < HCH else w1b[:, j - HCH, :, :]

            def w2_sb(j, k):
                return (w2a[:, j, k, :] if k < HCH
                        else w2b[:, j, k - HCH, :])

            def w3_sb(ot, k):
                return w3a[:, k, :] if ot == 0 else w3b[:, k, :]

            for bi, (t0, tb) in enumerate(blocks):
                xg_t = xpool.tile([P, DC, tb], BF16, tag="xg")
                q = nc.sync if bi % 2 == 0 else nc.scalar
                q.dma_start(
                    xg_t[:], xgb_ext[bi] if tb == TB else xgt_ext[:])
                if first_w1[0]:
                    # w1 rides sync AFTER block-0 x so L1 can start early
                    nc.sync.dma_start(w1a[:], w1_ext[:, 0:HCH, :, :])
                    nc.sync.dma_start(w1b[:], w1_ext[:, HCH:HC, :, :])
                    first_w1[0] = False

                # Accumulation chains are interleaved in PAIRS of PSUM banks
                # throughout: back-to-back accumulating matmuls into the SAME
                # bank stall the PE (write-read turnaround); alternating two
                # banks hides it.

                # layer 1: h1T[j] = relu(sum_dc W1t(j,dc).T @ xgT(dc) + b1)
                h1T = apool.tile([P, HC, tb], BF16, tag="hact")
                for jp in range(0, HC, 2):
                    pss = [mmps.tile([P, tb], F32, tag="mm", name=f"ps1{i}")
                           for i in range(2)]
                    for dc in range(DC):
                        for i in range(2):
                            nc.tensor.matmul(
                                pss[i][:], w1_sb(jp + i)[:, dc, :],
                                xg_t[:, dc, 0:tb],
                                start=(dc == 0), stop=(dc == DC - 1),
                            )
                    for i in range(2):
                        nc.scalar.activation(
                            h1T[:, jp + i, :], pss[i][:],
                            mybir.ActivationFunctionType.Relu,
                            bias=b1_sb[:, jp + i:jp + i + 1],
                        )

                # layer 2: h2T[j2] = relu(sum_k W2t(j2,k).T @ h1T(k) + b2)
                h2T = apool.tile([P, HC, tb], BF16, tag="hact")
                for jp in range(0, HC, 2):
                    pss = [mmps.tile([P, tb], F32, tag="mm", name=f"ps2{i}")
                           for i in range(2)]
                    for k in range(HC):
                        for i in range(2):
                            nc.tensor.matmul(
                                pss[i][:], w2_sb(jp + i, k),
                                h1T[:, k, :],
                                start=(k == 0), stop=(k == HC - 1),
                            )
                    for i in range(2):
                        nc.scalar.activation(
                            h2T[:, jp + i, :], pss[i][:],
                            mybir.ActivationFunctionType.Relu,
                            bias=b2_sb[:, jp + i:jp + i + 1],
                        )

                # layer 3 + routing-weight scale, 128-token chunks in pairs
                nq = tb // P
                for qp in range(0, nq, 2):
                    npair = min(2, nq - qp)
                    for ot in range(OT):
                        psOs = [outps.tile([P, OW], F32, tag="out",
                                           name=f"psO{i}")
                                for i in range(npair)]
                        for k in range(HC):
                            for i in range(npair):
                                qq = qp + i
                                nc.tensor.matmul(
                                    psOs[i][:],
                                    h2T[:, k, qq * P:(qq + 1) * P],
                                    w3_sb(ot, k),
                                    start=(k == 0), stop=(k == HC - 1),
                                )
                        for i in range(npair):
                            q2 = t0 // P + qp + i
                            yt = ypool.tile([P, OW], y_dt, tag="yt")
                            nc.vector.tensor_scalar_mul(
                                yt[:], psOs[i][:], wg_sb[:, q2:q2 + 1])
                            nc.gpsimd.dma_start(
                                y_ext[q2 * P:(q2 + 1) * P,
                                      ot * OW:(ot + 1) * OW],
                                yt[:])
    nc.compile()
    return nc


# ---------------------------------------------------------------------------
# Host side: routing, packing, PJRT runner (jit once, reusable), unshard.
# ---------------------------------------------------------------------------

def route(x, gate_W, gate_b):
    """fp64 gate; reproduces the reference's fp32 top-5 selection exactly
    (verified: min |p5-p6| gap 1.9e-6 >> fp32 matmul noise ~1e-7)."""
    s = (x.astype(np.float64) @ gate_W.astype(np.float64)
         + gate_b.astype(np.float64)) / TEMP
    s -= s.max(axis=-1, keepdims=True)
    p = np.exp(s)
    p /= p.sum(axis=-1, keepdims=True)
    order = np.argsort(-p, axis=-1, kind="stable")   # jax top_k tie-break
    mask = np.zeros_like(p)
    mask[np.arange(p.shape[0])[:, None], order[:, :N_ACTIVE]] = 1.0
    w = p * mask
    w /= (w.sum(axis=-1, keepdims=True) + EPS)
    return w.astype(np.float32)


def pack_inputs(x, gate_W, gate_b, W1, b1, W2, b2, W3, b3, cap=C_DEFAULT):
    """Route on host; per expert e, gather + pre-tile its tokens/weights."""
    P = 128
    n_experts, d, h = W1.shape
    o = W3.shape[2]
    DC, HC = d // P, h // P
    OW = 512
    OT = o // OW
    NQ = cap // P
    f32 = np.float32

    import ml_dtypes
    bf16 = ml_dtypes.bfloat16

    w = route(x, gate_W, gate_b)                      # [N, E]
    in_maps, aux = [], []
    for e in range(n_experts):
        idx = np.nonzero(w[:, e])[0]
        cnt = len(idx)
        assert cnt <= cap, f"expert {e}: {cnt} active tokens > cap {cap}"
        xg = np.zeros((cap, d), f32)
        xg[:cnt] = x[idx]
        # pre-blocked layouts (see build_nc): xgb[b,p,dc,t] = xg[b*TB+t, dc*128+p]
        TB = 512
        NBF, TBT = cap // TB, cap % TB
        v = xg.reshape(cap, DC, P)
        xgb = np.ascontiguousarray(
            v[:NBF * TB].reshape(NBF, TB, DC, P).transpose(0, 3, 2, 1)
        ).astype(bf16)
        xgt = (np.ascontiguousarray(
            v[NBF * TB:].transpose(2, 1, 0)).astype(bf16) if TBT else None)
        wgv = np.zeros((cap,), f32)
        wgv[:cnt] = w[idx, e]
        wgp = np.ascontiguousarray(wgv.reshape(NQ, P).T)      # [P, NQ]
        # (p, j, dc, ph) = W1[e, dc*128+p, j*128+ph]
        w1p = np.ascontiguousarray(
            W1[e].reshape(DC, P, HC, P).transpose(1, 2, 0, 3)).astype(bf16)
        # w2q[p, half, j, kk, q] = W2[e, (half*8+kk)*128+p, j*128+q]
        w2q = np.ascontiguousarray(
            W2[e].reshape(2, HC // 2, P, HC, P)    # (half, kk, p, j, q)
            .transpose(2, 0, 3, 1, 4)).astype(bf16)  # (p, half, j, kk, q)
        # (p, ot, k, ow) = W3[e, k*128+p, ot*512+ow]
        w3p = np.ascontiguousarray(
            W3[e].reshape(HC, P, OT, OW).transpose(1, 2, 0, 3)).astype(bf16)
        b1p = np.ascontiguousarray(b1[e].reshape(HC, P).T).astype(f32)
        b2p = np.ascontiguousarray(b2[e].reshape(HC, P).T).astype(f32)
        m = dict(xgb=xgb, w1p=w1p, w2q=w2q, w3p=w3p,
                 b1p=b1p, b2p=b2p, wg=wgp)
        if xgt is not None:
            m["xgt"] = xgt
        in_maps.append(m)
        aux.append((idx, cnt))
    return in_maps, aux, w


def unshard(res, aux, w, b3, o=O):
    """Scatter-add disjoint per-expert outputs; add the w @ b3 term."""
    y = np.zeros((N, o), np.float32)
    for e, (idx, cnt) in enumerate(aux):
        y[idx] += res[e]["y"][:cnt].astype(np.float32)
    y += w @ b3.astype(np.float32)                    # b3 is [E, O]
    return y


class SpmdRunner:
    """jit-once PJRT SPMD runner (mirrors bass2jax.run_bass_via_pjrt but
    reusable across calls so the NEFF compile is paid once)."""

    def __init__(self, nc, n_cores):
        import jax
        from jax.sharding import Mesh, PartitionSpec
        from jax.experimental.shard_map import shard_map
        from concourse import bass2jax as b2j

        b2j.install_neuronx_cc_hook()
        self.nc = nc
        self.n_cores = n_cores
        in_names, out_names, out_avals, zero_outs = [], [], [], []
        for alloc in nc.m.functions[0].allocations:
            if not isinstance(alloc, mybir.MemoryLocationSet):
                continue
            name = alloc.memorylocations[0].name
            if alloc.kind == "ExternalInput":
                if not (nc.partition_id_tensor
                        and name == nc.partition_id_tensor.name):
                    in_names.append(name)
            elif alloc.kind == "ExternalOutput":
                out_names.append(name)
                shape = tuple(alloc.tensor_shape)
                dtype = mybir.dt.np(alloc.dtype)
                out_avals.append(jax.core.ShapedArray(shape, dtype))
                zero_outs.append(np.zeros(shape, dtype))
        self.in_names, self.out_names = in_names, out_names
        self.out_avals, self.zero_outs = out_avals, zero_outs
        n_params, n_outs = len(in_names), len(out_names)
        self.n_params = n_params
        all_in_names = list(in_names) + list(out_names)
        partition_name = (nc.partition_id_tensor.name
                          if nc.partition_id_tensor else None)
        if partition_name is not None:
            all_in_names.append(partition_name)

        def _body(*args):
            operands = list(args)
            if partition_name is not None:
                operands.append(b2j.partition_id_tensor())
            outs = b2j._bass_exec_p.bind(
                *operands,
                out_avals=tuple(out_avals),
                in_names=tuple(all_in_names),
                out_names=tuple(out_names),
                lowering_input_output_aliases=(),
                sim_require_finite=True,
                sim_require_nnan=True,
                nc=nc,
            )
            return tuple(outs)

        devices = jax.devices()[:n_cores]
        self.mesh = Mesh(np.asarray(devices), ("core",))
        in_specs = (PartitionSpec("core"),) * (n_params + n_outs)
        out_specs = (PartitionSpec("core"),) * n_outs
        donate = tuple(range(n_params, n_params + n_outs))
        self.fn = jax.jit(
            shard_map(_body, mesh=self.mesh, in_specs=in_specs,
                      out_specs=out_specs, check_rep=False),
            donate_argnums=donate, keep_unused=True,
        )
        # non-donating variant for repeated timing (zeros uploaded once,
        # outputs allocated fresh each exec -> every exec is observable)
        self.fn_nodonate = jax.jit(
            shard_map(_body, mesh=self.mesh, in_specs=in_specs,
                      out_specs=out_specs, check_rep=False),
            keep_unused=True,
        )
        self.jax = jax

    def prep(self, in_maps):
        concat = [
            np.concatenate([np.asarray(m[n]) for m in in_maps], axis=0)
            for n in self.in_names
        ]
        return concat

    def zeros(self):
        return [np.zeros((self.n_cores * z.shape[0], *z.shape[1:]), z.dtype)
                for z in self.zero_outs]

    def __call__(self, concat_in, concat_zeros):
        out = self.fn(*concat_in, *concat_zeros)
        self.jax.block_until_ready(out)
        return out

    def time_batches(self, concat_in, rounds=8, small=1, big=33):
        """Per-exec device time via differential batches on the
        non-donating fn: per = (min T_big - min T_small)/(big - small).
        Each timed batch is preceded by an untimed absorber exec (soaks
        executable-switch and queue-warm costs)."""
        import time as _time
        import jax
        from jax.sharding import NamedSharding, PartitionSpec
        sh = NamedSharding(self.mesh, PartitionSpec("core"))
        darrs = [jax.device_put(a, sh) for a in concat_in]
        jax.block_until_ready(darrs)
        zeros = [jax.device_put(z, sh) for z in self.zeros()]
        jax.block_until_ready(zeros)
        o = self.fn_nodonate(*darrs, *zeros)
        jax.block_until_ready(o)
        del o

        def batch(k):
            o = self.fn_nodonate(*darrs, *zeros)     # absorber
            jax.block_until_ready(o)
            del o
            outs = []
            t0 = _time.perf_counter()
            for _ in range(k):
                outs.append(self.fn_nodonate(*darrs, *zeros))
            jax.block_until_ready(outs)
            dt = _time.perf_counter() - t0
            del outs
            return dt

        ts, tb = [], []
        for _ in range(rounds):
            ts.append(batch(small))
            tb.append(batch(big))
        per = (min(tb) - min(ts)) / (big - small)
        return per, ts, tb

    def split_outs(self, out_arrs):
        res = []
        for c in range(self.n_cores):
            res.append({
                name: np.asarray(out_arrs[i]).reshape(
                    self.n_cores, *self.out_avals[i].shape)[c]
                for i, name in enumerate(self.out_names)
            })
        return res


_CACHE = {}


def _get_runner(cap=C_DEFAULT):
    if cap not in _CACHE:
        nc = build_nc(cap=cap)
        _CACHE[cap] = SpmdRunner(nc, CORES)
    return _CACHE[cap]


def kernel(**inputs):
    x = np.asarray(inputs["x"], dtype=np.float32)
    gate_W = np.asarray(inputs["gate_W"], dtype=np.float32)
    gate_b = np.asarray(inputs["gate_b"], dtype=np.float32)
    W1 = np.asarray(inputs["W1"], dtype=np.float32)
    b1 = np.asarray(inputs["b1"], dtype=np.float32)
    W2 = np.asarray(inputs["W2"], dtype=np.float32)
    b2 = np.asarray(inputs["b2"], dtype=np.float32)
    W3 = np.asarray(inputs["W3"], dtype=np.float32)
    b3 = np.asarray(inputs["b3"], dtype=np.float32)

    # capacity: fixed 5248 covers the reference data (max 5156); if some
    # other input needs more, rebuild at the next 128-multiple.
    w = route(x, gate_W, gate_b)
    maxcnt = int((w > 0).sum(axis=0).max())
    cap = C_DEFAULT if maxcnt <= C_DEFAULT else ((maxcnt + 127) // 128) * 128
    runner = _get_runner(cap)
    in_maps, aux, w = pack_inputs(x, gate_W, gate_b, W1, b1, W2, b2, W3, b3,
                                  cap=cap)
    out = runner(runner.prep(in_maps), runner.zeros())
    res = runner.split_outs(out)
    return unshard(res, aux, w, b3).astype(np.float32)


if __name__ == "__main__":
    print("building...")
    nc = build_nc()
    print("built ok")
